# revision 1
# baseline (speedup 1.0000x reference)
"""GCN layer (SpMM + linear) on 8 Trainium2 NeuronCores.

out[i] = (sum_{e: edge_row[e]==i} edge_val[e] * x[edge_col[e]]) @ W.T + b

Strategy (per sharding hint): destination rows are partitioned across the 8
cores (6250 rows each).  Each core holds a full replica of x in its own HBM, so
cross-partition source rows are fetched with local dma_gather -- no
collectives.

Per-core device algorithm:
  - Destination rows are processed in fixed 16-row windows.  Each window owns
    two 128-slot gather tiles: one for edges whose source column is < 25000
    ("lo"), one for the rest ("hi") -- dma_gather uses int16 indices, so each
    tile gathers from one 25000-row half of x.  Slot p of a tile fetches one
    512B row of x via the GPSIMD dma_gather fast path (num_idxs batched per
    SWDGE call).
  - A host-precomputed selection matrix sval [128 slots, 16 rows] holds
    edge_val at (slot, local_row).  matmul(lhsT=msgs, rhs=sval) accumulates
    agg.T[feat, row] into PSUM -- the segment-sum runs on the TensorEngine.
  - Windows whose lo (or hi) edges exceed 128 spill the excess to per-group
    overflow tiles; their [128, 496] selection matrix is built on-device with
    one DVE tensor_scalar ((iota == local_row) * val) and accumulated with
    start=False.
  - Epilogue per 496-row PSUM group: copy agg.T to SBUF, matmul with W.T, add
    bias, DMA out.

Everything is fp32 end-to-end; PSUM accumulates in fp32.
"""

import math
from dataclasses import dataclass

import numpy as np


@dataclass(frozen=True)
class Cfg:
    n_nodes: int = 50000
    n_edges: int = 800000
    d: int = 128
    n_cores: int = 8
    win_rows: int = 16        # dest rows per window (one lo + one hi tile)
    group_wins: int = 31      # windows per PSUM group (496 rows = 1 bank)
    chunk: int = 32           # tiles per dma_gather call
    repeats: int = 1          # unrolled body repetitions (timing only)
    loop_n: int = 0           # if >0, wrap body in a For_i hardware loop
    fake_gather: bool = False  # timing diagnostic: sequential DMA, wrong data
    gather_elem: int = 128     # elems per gather descriptor (128 = correct)
    gather_queues: int = 1     # SWDGE queues (2 = lo/hi on separate queues)
    gather_bufs: int = 2       # gather buffer double/triple buffering

    @property
    def split(self) -> int:
        # lo/hi source split point; both halves must stay below 32768 rows
        # (int16 index range) and should carry ~equal edge probability.
        return self.n_nodes // 2

    @property
    def rows_per_core(self) -> int:
        return self.n_nodes // self.n_cores

    @property
    def n_wins(self) -> int:
        return math.ceil(self.rows_per_core / self.win_rows)

    @property
    def n_groups(self) -> int:
        return math.ceil(self.n_wins / self.group_wins)

    @property
    def win_counts(self) -> list:
        full = [self.group_wins] * (self.n_groups - 1)
        return full + [self.n_wins - self.group_wins * (self.n_groups - 1)]

    @property
    def group_rows(self) -> int:
        return self.group_wins * self.win_rows

    @property
    def out_rows(self) -> int:
        return self.n_wins * self.win_rows


@dataclass(frozen=True)
class Plan:
    ovf_lo: tuple  # overflow tiles per group, lo stream (max across cores)
    ovf_hi: tuple

    def t_stream(self, cfg: Cfg, ovf) -> int:
        return cfg.n_wins + sum(ovf)

    def t_pad(self, cfg: Cfg) -> int:
        t = max(self.t_stream(cfg, self.ovf_lo), self.t_stream(cfg, self.ovf_hi))
        return math.ceil(t / cfg.chunk) * cfg.chunk

    def ovt(self) -> int:
        return max(sum(self.ovf_lo), sum(self.ovf_hi), 1)


def _preprocess(cfg: Cfg, x, edge_row, edge_col, edge_val, W, b):
    """Bucket edges by (16-row window, lo/hi source half); build per-core
    gather-index and selection-value arrays.  Returns (plan, in_maps)."""
    RPC = cfg.rows_per_core
    WR = cfg.win_rows
    NW = cfg.n_wins
    NG = cfg.n_groups
    GW = cfg.group_wins
    GR = cfg.group_rows
    SPLIT = cfg.split
    CAP = 128
    D = cfg.d

    edge_row = np.asarray(edge_row)
    edge_col = np.asarray(edge_col)
    edge_val = np.asarray(edge_val)

    win_bounds_local = np.minimum(np.arange(NW + 1) * WR, RPC)

    # Pass 1: per (core, window, src) edge lists and overflow tile counts.
    per_core = []
    ovf_tiles = np.zeros((cfg.n_cores, NG, 2), dtype=np.int64)
    for c in range(cfg.n_cores):
        bounds = c * RPC + win_bounds_local
        estart = np.searchsorted(edge_row, bounds)
        e0, e1 = estart[0], estart[-1]
        idx = np.arange(e0, e1)
        r_loc = edge_row[e0:e1] - c * RPC
        w_of_e = np.minimum(r_loc // WR, NW - 1)
        hi = (edge_col[e0:e1] >= SPLIT).astype(np.int64)
        # rank of each edge within its (window, src) bucket
        key = w_of_e * 2 + hi
        order = np.argsort(key, kind="stable")
        inv = np.empty_like(order)
        inv[order] = np.arange(len(order))
        sk = key[order]
        bucket_start = np.searchsorted(sk, np.arange(NW * 2))
        pos = inv - bucket_start[key]
        is_main = pos < CAP
        g_of_e = w_of_e // GW
        per_core.append((idx, r_loc, w_of_e, hi, pos, is_main, g_of_e))
        for g in range(NG):
            for s in range(2):
                n_ovf = int(np.count_nonzero(~is_main & (g_of_e == g) & (hi == s)))
                ovf_tiles[c, g, s] = math.ceil(n_ovf / CAP)

    plan = Plan(
        ovf_lo=tuple(int(v) for v in ovf_tiles[:, :, 0].max(axis=0)),
        ovf_hi=tuple(int(v) for v in ovf_tiles[:, :, 1].max(axis=0)),
    )
    T_PAD = plan.t_pad(cfg)
    OVT = plan.ovt()
    wc = cfg.win_counts

    # Stream layout per source: per group, main tiles then overflow tiles.
    def bases(ovf):
        mb = np.zeros(NG, dtype=np.int64)
        ob = np.zeros(NG, dtype=np.int64)
        sb = np.zeros(NG, dtype=np.int64)
        acc = sacc = 0
        for g in range(NG):
            mb[g] = acc
            ob[g] = acc + wc[g]
            acc += wc[g] + ovf[g]
            sb[g] = sacc
            sacc += ovf[g]
        return mb, ob, sb

    mb_lo, ob_lo, sb_lo = bases(plan.ovf_lo)
    mb_hi, ob_hi, sb_hi = bases(plan.ovf_hi)
    mb = [mb_lo, mb_hi]
    ob = [ob_lo, ob_hi]
    sb = [sb_lo, sb_hi]

    # Shared constants: iot | bb | wt | (per-core ovr/ovv appended below).
    wt = np.asarray(W).T.astype(np.float32)
    bb = np.tile(np.asarray(b)[None, :].astype(np.float32), (128, 1))
    iot = np.tile(np.arange(GR, dtype=np.float32)[None, :], (128, 1))
    consts = np.concatenate([iot, bb, wt], axis=1).astype(np.float32)

    def wrap_idx(idx_lin):
        # dma_gather index layout: idxs_ap[p, s] = idx_lin[s*16 + p%16],
        # replicated across the eight 16-partition blocks.
        return np.tile(np.ascontiguousarray(idx_lin.reshape(-1, 16).T), (8, 1))

    in_maps = []
    for c in range(cfg.n_cores):
        idx, r_loc, w_of_e, hi, pos, is_main, g_of_e = per_core[c]
        idx_lin = [np.zeros(T_PAD * CAP, dtype=np.int16) for _ in range(2)]
        sval = [np.zeros((128, T_PAD * WR), dtype=np.float32) for _ in range(2)]
        ovr = [np.zeros((128, OVT), dtype=np.float32) for _ in range(2)]
        ovv = [np.zeros((128, OVT), dtype=np.float32) for _ in range(2)]

        for s in range(2):
            # Main edges.
            m = is_main & (hi == s)
            w_m = w_of_e[m]
            g_m = w_m // GW
            tile_pos = mb[s][g_m] + (w_m - g_m * GW)
            p_m = pos[m]
            col = edge_col[idx[m]] - s * SPLIT
            idx_lin[s][tile_pos * CAP + p_m] = col.astype(np.int16)
            sval[s][p_m, tile_pos * WR + (r_loc[m] - w_m * WR)] = edge_val[idx[m]]
            # Overflow edges per group.
            for g in range(NG):
                sel = (~is_main) & (g_of_e == g) & (hi == s)
                k = int(np.count_nonzero(sel))
                if k == 0:
                    continue
                ranks = np.arange(k)
                p_o = ranks % CAP
                tio = ranks // CAP
                tile_pos = ob[s][g] + tio
                seq = sb[s][g] + tio
                col = edge_col[idx[sel]] - s * SPLIT
                idx_lin[s][tile_pos * CAP + p_o] = col.astype(np.int16)
                ovr[s][p_o, seq] = (r_loc[sel] - g * GR).astype(np.float32)
                ovv[s][p_o, seq] = edge_val[idx[sel]]

        cst = np.ascontiguousarray(
            np.concatenate([consts, ovr[0], ovv[0], ovr[1], ovv[1]], axis=1),
            dtype=np.float32)
        in_maps.append({
            "x": np.ascontiguousarray(x, dtype=np.float32),
            "gilo": np.ascontiguousarray(wrap_idx(idx_lin[0])),
            "gihi": np.ascontiguousarray(wrap_idx(idx_lin[1])),
            "svlo": np.ascontiguousarray(sval[0]),
            "svhi": np.ascontiguousarray(sval[1]),
            "cst": cst,
        })
    return plan, in_maps


def _build_nc(cfg: Cfg, plan: Plan):
    from contextlib import ExitStack

    import concourse.bacc as bacc
    import concourse.mybir as mybir
    import concourse.tile as tile

    f32 = mybir.dt.float32
    i16 = mybir.dt.int16
    D = cfg.d
    WR = cfg.win_rows
    GR = cfg.group_rows
    NG = cfg.n_groups
    CH = cfg.chunk
    SPLIT = cfg.split
    T_PAD = plan.t_pad(cfg)
    OVT = plan.ovt()
    wc = cfg.win_counts

    # Packed constants: iot [GR] | bb [D] | wt [D] | ovr/ovv lo | ovr/ovv hi
    CW = GR + 2 * D + 4 * OVT
    O_IOT, O_BB, O_WT = 0, GR, GR + D
    O_OV = GR + 2 * D

    nc = bacc.Bacc("TRN2", target_bir_lowering=False,
                   num_swdge_queues=cfg.gather_queues)
    x_in = nc.dram_tensor("x", [cfg.n_nodes, D], f32, kind="ExternalInput")
    gi = [nc.dram_tensor("gilo", [128, T_PAD * 8], i16, kind="ExternalInput"),
          nc.dram_tensor("gihi", [128, T_PAD * 8], i16, kind="ExternalInput")]
    sv = [nc.dram_tensor("svlo", [128, T_PAD * WR], f32, kind="ExternalInput"),
          nc.dram_tensor("svhi", [128, T_PAD * WR], f32, kind="ExternalInput")]
    cst = nc.dram_tensor("cst", [128, CW], f32, kind="ExternalInput")
    y = nc.dram_tensor("y", [cfg.out_rows, D], f32, kind="ExternalOutput")

    x_src = [x_in[0:SPLIT, :], x_in[SPLIT:cfg.n_nodes, :]]

    with tile.TileContext(nc) as tc, ExitStack() as ctx:
        const = ctx.enter_context(tc.tile_pool(name="const", bufs=1))
        gpool = [ctx.enter_context(tc.tile_pool(name="glo", bufs=cfg.gather_bufs)),
                 ctx.enter_context(tc.tile_pool(name="ghi", bufs=cfg.gather_bufs))]
        opool = ctx.enter_context(tc.tile_pool(name="ovfsel", bufs=2))
        epool = ctx.enter_context(tc.tile_pool(name="epilog", bufs=2))
        ps_agg = ctx.enter_context(tc.tile_pool(name="psagg", bufs=2, space="PSUM"))
        ps_out = ctx.enter_context(tc.tile_pool(name="psout", bufs=2, space="PSUM"))

        cst_sb = const.tile([128, CW], f32)
        nc.sync.dma_start(out=cst_sb[:], in_=cst[:])

        def iot_ap(n):
            return cst_sb[:, O_IOT:O_IOT + n]

        def bb_ap(p):
            return cst_sb[:p, O_BB:O_BB + D]

        def wt_ap():
            return cst_sb[:, O_WT:O_WT + D]

        def ov_ap(s, which, q):
            o = O_OV + (2 * s + which) * OVT + q
            return cst_sb[:, o:o + 1]

        # Gather indices and selection values fully resident in SBUF.
        gi_sb = []
        sv_sb = []
        for s in range(2):
            t = const.tile([128, T_PAD * 8], i16, tag=f"gi{s}")
            nc.sync.dma_start(out=t[:], in_=gi[s][:])
            gi_sb.append(t)
            t = const.tile([128, T_PAD * WR], f32, tag=f"sv{s}")
            nc.sync.dma_start(out=t[:], in_=sv[s][:])
            sv_sb.append(t)

        gbuf = [None, None]

        def fetch_chunk(s, ci):
            gbuf[s] = gpool[s].tile([128, CH * D], f32, tag=f"gb{s}",
                                    name=f"gbuf{s}")
            if cfg.fake_gather:
                o = (s * 16384) % 32768
                nc.sync.dma_start(
                    out=gbuf[s][:],
                    in_=x_in[o:o + 128 * CH, :]
                    .rearrange("(p c) d -> p (c d)", p=128))
                return
            ge = cfg.gather_elem
            nc.gpsimd.dma_gather(
                gbuf[s][:, :CH * ge].rearrange("p (k j) -> p k j", j=ge),
                x_src[s] if ge == D else x_src[s].rearrange(
                    "a (c d) -> (a c) d", d=ge),
                gi_sb[s][:, ci * CH * 8:(ci + 1) * CH * 8],
                CH * 128, CH * 128, ge,
                # single-packet mode (needed for the fast CounterMachine DGE
                # path) caps one instruction at 1024 descriptors.
                single_packet=(CH * 128 <= 1024),
                queue_num=(s * 2 + ci % 2 if cfg.gather_queues == 4
                           else s % cfg.gather_queues),
            )

        def body():
            spos = [0, 0]   # per-stream tile position
            oseq = [0, 0]   # per-stream overflow sequence
            for g in range(NG):
                rows_g = wc[g] * WR
                agg = ps_agg.tile([128, GR], f32)
                # Main windows: lo tile then hi tile accumulate the same 16 cols.
                for i in range(wc[g]):
                    for s in range(2):
                        if spos[s] % CH == 0:
                            fetch_chunk(s, spos[s] // CH)
                        k = spos[s] % CH
                        nc.tensor.matmul(
                            out=agg[:, i * WR:(i + 1) * WR],
                            lhsT=gbuf[s][:, k * D:(k + 1) * D],
                            rhs=sv_sb[s][:, spos[s] * WR:(spos[s] + 1) * WR],
                            start=(i == 0 and s == 0), stop=True,
                            skip_group_check=True,
                        )
                        spos[s] += 1
                # Overflow tiles: on-device selection matrix, accumulate.
                for s in range(2):
                    for _ in range((plan.ovf_lo, plan.ovf_hi)[s][g]):
                        if spos[s] % CH == 0:
                            fetch_chunk(s, spos[s] // CH)
                        k = spos[s] % CH
                        osel = opool.tile([128, GR], f32)
                        nc.vector.tensor_scalar(
                            osel[:, :rows_g], iot_ap(rows_g),
                            ov_ap(s, 0, oseq[s]), ov_ap(s, 1, oseq[s]),
                            mybir.AluOpType.is_equal, mybir.AluOpType.mult,
                        )
                        nc.tensor.matmul(
                            out=agg[:, :rows_g],
                            lhsT=gbuf[s][:, k * D:(k + 1) * D],
                            rhs=osel[:, :rows_g],
                            start=False, stop=True,
                            skip_group_check=True,
                        )
                        spos[s] += 1
                        oseq[s] += 1
                # Epilogue: out[rows, dout] = agg.T @ W.T + b
                agg_sb = epool.tile([128, GR], f32, tag="aggsb")
                nc.vector.tensor_copy(out=agg_sb[:, :rows_g], in_=agg[:, :rows_g])
                for rc in range(math.ceil(rows_g / 128)):
                    w = min(128, rows_g - rc * 128)
                    out_ps = ps_out.tile([128, D], f32)
                    nc.tensor.matmul(
                        out=out_ps[:w, :],
                        lhsT=agg_sb[:, rc * 128:rc * 128 + w],
                        rhs=wt_ap(),
                        start=True, stop=True,
                    )
                    out_sb = epool.tile([128, D], f32, tag="outsb")
                    nc.vector.tensor_tensor(
                        out=out_sb[:w, :], in0=out_ps[:w, :], in1=bb_ap(w),
                        op=mybir.AluOpType.add,
                    )
                    r0 = g * GR + rc * 128
                    nc.sync.dma_start(out=y[r0:r0 + w, :], in_=out_sb[:w, :])

        if cfg.loop_n > 0:
            with tc.For_i(0, cfg.loop_n, 1):
                body()
        else:
            for _ in range(cfg.repeats):
                body()

    nc.compile()
    return nc


_CACHE = {}


def _get_nc(cfg: Cfg, plan: Plan):
    key = (cfg, plan)
    if key not in _CACHE:
        _CACHE[key] = _build_nc(cfg, plan)
    return _CACHE[key]


def kernel(x, edge_row, edge_col, edge_val, W, b):
    from concourse.bass_utils import run_bass_kernel_spmd

    cfg = Cfg()
    plan, in_maps = _preprocess(cfg, x, edge_row, edge_col, edge_val, W, b)
    nc = _get_nc(cfg, plan)
    res = run_bass_kernel_spmd(nc, in_maps, core_ids=list(range(cfg.n_cores)))
    RPC = cfg.rows_per_core
    out = np.empty((cfg.n_nodes, cfg.d), dtype=np.float32)
    for c in range(cfg.n_cores):
        out[c * RPC:(c + 1) * RPC] = res.results[c]["y"][:RPC]
    return out



# revision 9
# speedup vs baseline: 1.4096x; 1.4096x over previous
"""GCN layer (SpMM + linear) on 8 Trainium2 NeuronCores.

out[i] = (sum_{e: edge_row[e]==i} edge_val[e] * x[edge_col[e]]) @ W.T + b

Strategy (per sharding hint): destination rows are partitioned across the 8
cores (6250 rows each).  Each core holds a full replica of x in its own HBM, so
cross-partition source rows are fetched with local dma_gather -- no
collectives.

Per-core device algorithm:
  - Destination rows are processed in fixed 16-row windows.  Each window owns
    two 128-slot gather tiles: one for edges whose source column is < 25000
    ("lo"), one for the rest ("hi") -- dma_gather uses int16 indices, so each
    tile gathers from one 25000-row half of x.  Slot p of a tile fetches one
    512B row of x via the GPSIMD dma_gather fast path (num_idxs batched per
    SWDGE call).
  - A host-precomputed selection matrix sval [128 slots, 16 rows] holds
    edge_val at (slot, local_row).  matmul(lhsT=msgs, rhs=sval) accumulates
    agg.T[feat, row] into PSUM -- the segment-sum runs on the TensorEngine.
  - Windows whose lo (or hi) edges exceed 128 spill the excess to per-group
    overflow tiles; their [128, 496] selection matrix is built on-device with
    one DVE tensor_scalar ((iota == local_row) * val) and accumulated with
    start=False.
  - Epilogue per 496-row PSUM group: copy agg.T to SBUF, matmul with W.T, add
    bias, DMA out.

Everything is fp32 end-to-end; PSUM accumulates in fp32.
"""

import math
from dataclasses import dataclass

import numpy as np


@dataclass(frozen=True)
class Cfg:
    n_nodes: int = 50000
    n_edges: int = 800000
    d: int = 128
    n_cores: int = 8
    win_rows: int = 16        # dest rows per window (one lo + one hi tile)
    group_wins: int = 31      # windows per PSUM group (496 rows = 1 bank)
    chunk: int = 32           # tiles per dma_gather call
    repeats: int = 1          # unrolled body repetitions (timing only)
    loop_n: int = 0           # if >0, wrap body in a For_i hardware loop
    fake_gather: bool = False  # timing diagnostic: sequential DMA, wrong data
    skip_compute: bool = False  # timing diagnostic: gathers only, no matmuls
    idx_diag: str = "off"      # timing diagnostic: zero|tile_sort|stream_sort
    gather_elem: int = 128     # elems per gather descriptor (128 = correct)
    gather_queues: int = 1     # SWDGE queues (2 = lo/hi on separate queues)
    gather_bufs: int = 2       # gather buffer double/triple buffering

    @property
    def split(self) -> int:
        # lo/hi source split point; both halves must stay below 32768 rows
        # (int16 index range) and should carry ~equal edge probability.
        return self.n_nodes // 2

    @property
    def rows_per_core(self) -> int:
        return self.n_nodes // self.n_cores

    @property
    def n_wins(self) -> int:
        return math.ceil(self.rows_per_core / self.win_rows)

    @property
    def n_groups(self) -> int:
        return math.ceil(self.n_wins / self.group_wins)

    @property
    def win_counts(self) -> list:
        full = [self.group_wins] * (self.n_groups - 1)
        return full + [self.n_wins - self.group_wins * (self.n_groups - 1)]

    @property
    def group_rows(self) -> int:
        return self.group_wins * self.win_rows

    @property
    def out_rows(self) -> int:
        return self.n_wins * self.win_rows


@dataclass(frozen=True)
class Plan:
    ovf_lo: tuple  # overflow tiles per group, lo stream (max across cores)
    ovf_hi: tuple

    def t_stream(self, cfg: Cfg, ovf) -> int:
        return cfg.n_wins + sum(ovf)

    def t_pad(self, cfg: Cfg) -> int:
        t = max(self.t_stream(cfg, self.ovf_lo), self.t_stream(cfg, self.ovf_hi))
        return math.ceil(t / cfg.chunk) * cfg.chunk

    def ovt(self) -> int:
        return max(sum(self.ovf_lo), sum(self.ovf_hi), 1)


def _preprocess(cfg: Cfg, x, edge_row, edge_col, edge_val, W, b):
    """Bucket edges by (16-row window, lo/hi source half); build per-core
    gather-index and selection-value arrays.  Returns (plan, in_maps)."""
    RPC = cfg.rows_per_core
    WR = cfg.win_rows
    NW = cfg.n_wins
    NG = cfg.n_groups
    GW = cfg.group_wins
    GR = cfg.group_rows
    SPLIT = cfg.split
    CAP = 128
    D = cfg.d

    edge_row = np.asarray(edge_row)
    edge_col = np.asarray(edge_col)
    edge_val = np.asarray(edge_val)

    win_bounds_local = np.minimum(np.arange(NW + 1) * WR, RPC)

    # Pass 1: per (core, window, src) edge lists and overflow tile counts.
    per_core = []
    ovf_tiles = np.zeros((cfg.n_cores, NG, 2), dtype=np.int64)
    for c in range(cfg.n_cores):
        bounds = c * RPC + win_bounds_local
        estart = np.searchsorted(edge_row, bounds)
        e0, e1 = estart[0], estart[-1]
        idx = np.arange(e0, e1)
        r_loc = edge_row[e0:e1] - c * RPC
        w_of_e = np.minimum(r_loc // WR, NW - 1)
        hi = (edge_col[e0:e1] >= SPLIT).astype(np.int64)
        # rank of each edge within its (window, src) bucket
        key = w_of_e * 2 + hi
        order = np.argsort(key, kind="stable")
        inv = np.empty_like(order)
        inv[order] = np.arange(len(order))
        sk = key[order]
        bucket_start = np.searchsorted(sk, np.arange(NW * 2))
        pos = inv - bucket_start[key]
        is_main = pos < CAP
        g_of_e = w_of_e // GW
        per_core.append((idx, r_loc, w_of_e, hi, pos, is_main, g_of_e))
        for g in range(NG):
            for s in range(2):
                n_ovf = int(np.count_nonzero(~is_main & (g_of_e == g) & (hi == s)))
                ovf_tiles[c, g, s] = math.ceil(n_ovf / CAP)

    plan = Plan(
        ovf_lo=tuple(int(v) for v in ovf_tiles[:, :, 0].max(axis=0)),
        ovf_hi=tuple(int(v) for v in ovf_tiles[:, :, 1].max(axis=0)),
    )
    T_PAD = plan.t_pad(cfg)
    OVT = plan.ovt()
    wc = cfg.win_counts

    # Stream layout per source: per group, main tiles then overflow tiles.
    def bases(ovf):
        mb = np.zeros(NG, dtype=np.int64)
        ob = np.zeros(NG, dtype=np.int64)
        sb = np.zeros(NG, dtype=np.int64)
        acc = sacc = 0
        for g in range(NG):
            mb[g] = acc
            ob[g] = acc + wc[g]
            acc += wc[g] + ovf[g]
            sb[g] = sacc
            sacc += ovf[g]
        return mb, ob, sb

    mb_lo, ob_lo, sb_lo = bases(plan.ovf_lo)
    mb_hi, ob_hi, sb_hi = bases(plan.ovf_hi)
    mb = [mb_lo, mb_hi]
    ob = [ob_lo, ob_hi]
    sb = [sb_lo, sb_hi]

    # Shared constants: iot | bb | wt | (per-core ovr/ovv appended below).
    wt = np.asarray(W).T.astype(np.float32)
    bb = np.tile(np.asarray(b)[None, :].astype(np.float32), (128, 1))
    iot = np.tile(np.arange(GR, dtype=np.float32)[None, :], (128, 1))
    consts = np.concatenate([iot, bb, wt], axis=1).astype(np.float32)

    def wrap_idx(idx_lin):
        # dma_gather index layout: idxs_ap[p, s] = idx_lin[s*16 + p%16],
        # replicated across the eight 16-partition blocks.
        return np.tile(np.ascontiguousarray(idx_lin.reshape(-1, 16).T), (8, 1))

    in_maps = []
    for c in range(cfg.n_cores):
        idx, r_loc, w_of_e, hi, pos, is_main, g_of_e = per_core[c]
        idx_lin = [np.zeros(T_PAD * CAP, dtype=np.int16) for _ in range(2)]
        sval = [np.zeros((128, T_PAD * WR), dtype=np.float32) for _ in range(2)]
        ovr = [np.zeros((128, OVT), dtype=np.float32) for _ in range(2)]
        ovv = [np.zeros((128, OVT), dtype=np.float32) for _ in range(2)]

        for s in range(2):
            # Main edges.
            m = is_main & (hi == s)
            w_m = w_of_e[m]
            g_m = w_m // GW
            tile_pos = mb[s][g_m] + (w_m - g_m * GW)
            p_m = pos[m]
            col = edge_col[idx[m]] - s * SPLIT
            idx_lin[s][tile_pos * CAP + p_m] = col.astype(np.int16)
            sval[s][p_m, tile_pos * WR + (r_loc[m] - w_m * WR)] = edge_val[idx[m]]
            # Overflow edges per group.
            for g in range(NG):
                sel = (~is_main) & (g_of_e == g) & (hi == s)
                k = int(np.count_nonzero(sel))
                if k == 0:
                    continue
                ranks = np.arange(k)
                p_o = ranks % CAP
                tio = ranks // CAP
                tile_pos = ob[s][g] + tio
                seq = sb[s][g] + tio
                col = edge_col[idx[sel]] - s * SPLIT
                idx_lin[s][tile_pos * CAP + p_o] = col.astype(np.int16)
                ovr[s][p_o, seq] = (r_loc[sel] - g * GR).astype(np.float32)
                ovv[s][p_o, seq] = edge_val[idx[sel]]

        if cfg.idx_diag != "off":
            for s in range(2):
                il = idx_lin[s]
                if cfg.idx_diag == "zero":
                    il[:] = 0
                elif cfg.idx_diag == "half":
                    il[:] = il // 2
                elif cfg.idx_diag == "quarter":
                    il[:] = il // 4
                elif cfg.idx_diag == "tile_sort":
                    il[:] = np.sort(il.reshape(-1, CAP), axis=1).reshape(-1)
                elif cfg.idx_diag == "stream_sort":
                    il[:] = np.sort(il)
                else:
                    raise ValueError(cfg.idx_diag)
        cst = np.ascontiguousarray(
            np.concatenate([consts, ovr[0], ovv[0], ovr[1], ovv[1]], axis=1),
            dtype=np.float32)
        in_maps.append({
            "x": np.ascontiguousarray(x, dtype=np.float32),
            "gilo": np.ascontiguousarray(wrap_idx(idx_lin[0])),
            "gihi": np.ascontiguousarray(wrap_idx(idx_lin[1])),
            "svlo": np.ascontiguousarray(sval[0]),
            "svhi": np.ascontiguousarray(sval[1]),
            "cst": cst,
        })
    return plan, in_maps


def _build_nc(cfg: Cfg, plan: Plan):
    from contextlib import ExitStack

    import concourse.bacc as bacc
    import concourse.mybir as mybir
    import concourse.tile as tile

    f32 = mybir.dt.float32
    i16 = mybir.dt.int16
    D = cfg.d
    WR = cfg.win_rows
    GR = cfg.group_rows
    NG = cfg.n_groups
    CH = cfg.chunk
    SPLIT = cfg.split
    T_PAD = plan.t_pad(cfg)
    OVT = plan.ovt()
    wc = cfg.win_counts

    # Packed constants: iot [GR] | bb [D] | wt [D] | ovr/ovv lo | ovr/ovv hi
    CW = GR + 2 * D + 4 * OVT
    O_IOT, O_BB, O_WT = 0, GR, GR + D
    O_OV = GR + 2 * D

    nc = bacc.Bacc("TRN2", target_bir_lowering=False,
                   num_swdge_queues=cfg.gather_queues)
    x_in = nc.dram_tensor("x", [cfg.n_nodes, D], f32, kind="ExternalInput")
    gi = [nc.dram_tensor("gilo", [128, T_PAD * 8], i16, kind="ExternalInput"),
          nc.dram_tensor("gihi", [128, T_PAD * 8], i16, kind="ExternalInput")]
    sv = [nc.dram_tensor("svlo", [128, T_PAD * WR], f32, kind="ExternalInput"),
          nc.dram_tensor("svhi", [128, T_PAD * WR], f32, kind="ExternalInput")]
    cst = nc.dram_tensor("cst", [128, CW], f32, kind="ExternalInput")
    y = nc.dram_tensor("y", [cfg.out_rows, D], f32, kind="ExternalOutput")

    x_src = [x_in[0:SPLIT, :], x_in[SPLIT:cfg.n_nodes, :]]

    with tile.TileContext(nc) as tc, ExitStack() as ctx:
        const = ctx.enter_context(tc.tile_pool(name="const", bufs=1))
        gpool = [ctx.enter_context(tc.tile_pool(name="glo", bufs=cfg.gather_bufs)),
                 ctx.enter_context(tc.tile_pool(name="ghi", bufs=cfg.gather_bufs))]
        opool = ctx.enter_context(tc.tile_pool(name="ovfsel", bufs=2))
        epool = ctx.enter_context(tc.tile_pool(name="epilog", bufs=2))
        ps_agg = ctx.enter_context(tc.tile_pool(name="psagg", bufs=2, space="PSUM"))
        ps_out = ctx.enter_context(tc.tile_pool(name="psout", bufs=2, space="PSUM"))

        cst_sb = const.tile([128, CW], f32)
        nc.sync.dma_start(out=cst_sb[:], in_=cst[:])

        def iot_ap(n):
            return cst_sb[:, O_IOT:O_IOT + n]

        def bb_ap(p):
            return cst_sb[:p, O_BB:O_BB + D]

        def wt_ap():
            return cst_sb[:, O_WT:O_WT + D]

        def ov_ap(s, which, q):
            o = O_OV + (2 * s + which) * OVT + q
            return cst_sb[:, o:o + 1]

        # Gather indices and selection values fully resident in SBUF.
        gi_sb = []
        sv_sb = []
        for s in range(2):
            t = const.tile([128, T_PAD * 8], i16, tag=f"gi{s}")
            nc.sync.dma_start(out=t[:], in_=gi[s][:])
            gi_sb.append(t)
            t = const.tile([128, T_PAD * WR], f32, tag=f"sv{s}")
            nc.sync.dma_start(out=t[:], in_=sv[s][:])
            sv_sb.append(t)

        gbuf = [None, None]

        def fetch_chunk(s, ci):
            gbuf[s] = gpool[s].tile([128, CH * max(D, cfg.gather_elem)], f32,
                                    tag=f"gb{s}", name=f"gbuf{s}")
            if cfg.fake_gather:
                o = (s * 16384) % 32768
                nc.sync.dma_start(
                    out=gbuf[s][:],
                    in_=x_in[o:o + 128 * CH, :]
                    .rearrange("(p c) d -> p (c d)", p=128))
                return
            ge = cfg.gather_elem
            if ge == D:
                src = x_src[s]
            elif ge < D:
                src = x_src[s].rearrange("a (c d) -> (a c) d", d=ge)
            else:
                src = x_src[s].rearrange("(a c) d -> a (c d)", c=ge // D)
            nc.gpsimd.dma_gather(
                gbuf[s][:, :CH * ge].rearrange("p (k j) -> p k j", j=ge),
                src,
                gi_sb[s][:, ci * CH * 8:(ci + 1) * CH * 8],
                CH * 128, CH * 128, ge,
                # single-packet mode (needed for the fast CounterMachine DGE
                # path) caps one instruction at 1024 descriptors.
                single_packet=(CH * 128 <= 1024),
                queue_num=(s * 2 + ci % 2 if cfg.gather_queues == 4
                           else s % cfg.gather_queues),
            )

        def body():
            if cfg.skip_compute:
                n_ch = T_PAD // CH
                for ci in range(n_ch):
                    for s in range(2):
                        fetch_chunk(s, ci)
                return
            spos = [0, 0]   # per-stream tile position
            oseq = [0, 0]   # per-stream overflow sequence
            for g in range(NG):
                rows_g = wc[g] * WR
                agg = ps_agg.tile([128, GR], f32)
                # Main windows: lo tile then hi tile accumulate the same 16 cols.
                for i in range(wc[g]):
                    for s in range(2):
                        if spos[s] % CH == 0:
                            fetch_chunk(s, spos[s] // CH)
                        k = spos[s] % CH
                        nc.tensor.matmul(
                            out=agg[:, i * WR:(i + 1) * WR],
                            lhsT=gbuf[s][:, k * D:(k + 1) * D],
                            rhs=sv_sb[s][:, spos[s] * WR:(spos[s] + 1) * WR],
                            start=(i == 0 and s == 0), stop=True,
                            skip_group_check=True,
                        )
                        spos[s] += 1
                # Overflow tiles: on-device selection matrix, accumulate.
                for s in range(2):
                    for _ in range((plan.ovf_lo, plan.ovf_hi)[s][g]):
                        if spos[s] % CH == 0:
                            fetch_chunk(s, spos[s] // CH)
                        k = spos[s] % CH
                        osel = opool.tile([128, GR], f32)
                        nc.vector.tensor_scalar(
                            osel[:, :rows_g], iot_ap(rows_g),
                            ov_ap(s, 0, oseq[s]), ov_ap(s, 1, oseq[s]),
                            mybir.AluOpType.is_equal, mybir.AluOpType.mult,
                        )
                        nc.tensor.matmul(
                            out=agg[:, :rows_g],
                            lhsT=gbuf[s][:, k * D:(k + 1) * D],
                            rhs=osel[:, :rows_g],
                            start=False, stop=True,
                            skip_group_check=True,
                        )
                        spos[s] += 1
                        oseq[s] += 1
                # Epilogue: out[rows, dout] = agg.T @ W.T + b
                agg_sb = epool.tile([128, GR], f32, tag="aggsb")
                nc.vector.tensor_copy(out=agg_sb[:, :rows_g], in_=agg[:, :rows_g])
                for rc in range(math.ceil(rows_g / 128)):
                    w = min(128, rows_g - rc * 128)
                    out_ps = ps_out.tile([128, D], f32)
                    nc.tensor.matmul(
                        out=out_ps[:w, :],
                        lhsT=agg_sb[:, rc * 128:rc * 128 + w],
                        rhs=wt_ap(),
                        start=True, stop=True,
                    )
                    out_sb = epool.tile([128, D], f32, tag="outsb")
                    nc.vector.tensor_tensor(
                        out=out_sb[:w, :], in0=out_ps[:w, :], in1=bb_ap(w),
                        op=mybir.AluOpType.add,
                    )
                    r0 = g * GR + rc * 128
                    nc.sync.dma_start(out=y[r0:r0 + w, :], in_=out_sb[:w, :])

        if cfg.loop_n > 0:
            with tc.For_i(0, cfg.loop_n, 1):
                body()
        else:
            for _ in range(cfg.repeats):
                body()

    nc.compile()
    return nc


_CACHE = {}


def _get_nc(cfg: Cfg, plan: Plan):
    key = (cfg, plan)
    if key not in _CACHE:
        _CACHE[key] = _build_nc(cfg, plan)
    return _CACHE[key]


def kernel(x, edge_row, edge_col, edge_val, W, b):
    from concourse.bass_utils import run_bass_kernel_spmd

    cfg = Cfg()
    plan, in_maps = _preprocess(cfg, x, edge_row, edge_col, edge_val, W, b)
    nc = _get_nc(cfg, plan)
    res = run_bass_kernel_spmd(nc, in_maps, core_ids=list(range(cfg.n_cores)))
    RPC = cfg.rows_per_core
    out = np.empty((cfg.n_nodes, cfg.d), dtype=np.float32)
    for c in range(cfg.n_cores):
        out[c * RPC:(c + 1) * RPC] = res.results[c]["y"][:RPC]
    return out



# revision 10
# speedup vs baseline: 4.2466x; 3.0127x over previous
"""GCN layer (SpMM + linear) on 8 Trainium2 NeuronCores — exact-packed dest-banded tiles, host-dense selection (bf16).

out[i] = (sum_{e: edge_row[e]==i} edge_val[e] * x[edge_col[e]]) @ W.T + b

Destination rows are partitioned across 8 cores (6250 each) into 13 PSUM
groups of 496 rows.  Per (group, source-half) bucket, edges are sorted by
destination and packed 128 per gather tile — no per-window padding.  Tile t's
destinations fall in a narrow data-derived band [db[t], db[t]+BW); the
selection matrix sval[slot, dest-db] (bf16, host-precomputed, resident in
SBUF) is dense over the band, so duplicate (src,dst) edges just sum.

Each slot gathers one 256B bf16 x row via SWDGE dma_gather (int16 indices,
lo/hi source halves on separate queues).  matmul(lhsT=gathered, rhs=sval
band) accumulates agg.T[feat, dest] into the group's PSUM bank.  Epilogue per
group: copy to SBUF, project with W.T (fp32), add bias, DMA out.
"""

import math
from dataclasses import dataclass

import numpy as np

GR = 496          # dest rows per PSUM group
CAP = 128         # slots per gather tile
D = 128           # feature dim


@dataclass(frozen=True)
class Cfg:
    n_nodes: int = 50000
    n_edges: int = 800000
    n_cores: int = 8
    chunk: int = 8            # tiles per dma_gather call
    gather_queues: int = 4
    gather_bufs: int = 8
    skip_compute: bool = False
    loop_n: int = 0
    repeats: int = 1

    @property
    def split(self) -> int:
        return self.n_nodes // 2

    @property
    def rows_per_core(self) -> int:
        return self.n_nodes // self.n_cores

    @property
    def n_groups(self) -> int:
        return math.ceil(self.rows_per_core / GR)


@dataclass(frozen=True)
class Plan:
    tc: tuple    # tc[g][s]: tiles per (group, stream), max over cores
    db: tuple    # db[g][s][t]: band start column of tile t
    bw: int      # band width (compiled free size of the sval matmul)

    def t_stream(self, s):
        return sum(t[s] for t in self.tc)


def _preprocess(cfg: Cfg, x, edge_row, edge_col, edge_val, W, b):
    import ml_dtypes

    RPC = cfg.rows_per_core
    NG = cfg.n_groups
    SPLIT = cfg.split

    x = np.asarray(x)
    edge_row = np.asarray(edge_row)
    edge_col = np.asarray(edge_col)
    edge_val = np.asarray(edge_val)

    xb = np.ascontiguousarray(x.astype(ml_dtypes.bfloat16))
    xplo = np.ascontiguousarray(xb[:SPLIT])
    xphi = np.ascontiguousarray(xb[SPLIT:])

    # Pass 1: per (core, group, stream) dest-sorted edge arrays.
    per_core = []
    for c in range(cfg.n_cores):
        e0, e1 = np.searchsorted(edge_row, [c * RPC, (c + 1) * RPC])
        r_loc = edge_row[e0:e1] - c * RPC
        g_of_e = r_loc // GR
        d_loc = r_loc - g_of_e * GR
        src = edge_col[e0:e1].astype(np.int64)
        s_of_e = (src >= SPLIT).astype(np.int64)
        val = edge_val[e0:e1].astype(np.float64)
        buckets = {}
        for g in range(NG):
            for s in range(2):
                m = (g_of_e == g) & (s_of_e == s)
                order = np.argsort(d_loc[m], kind="stable")
                buckets[(g, s)] = (src[m][order] - s * SPLIT,
                                  d_loc[m][order], val[m][order])
        per_core.append(buckets)

    # Plan: tile counts, band starts, band width (shared across cores).
    tc = []
    for g in range(NG):
        tc.append(tuple(
            max(math.ceil(len(pc[(g, s)][0]) / CAP) for pc in per_core)
            for s in range(2)))

    db = []
    bw = 0
    for g in range(NG):
        row = []
        for s in range(2):
            nt = tc[g][s]
            starts = np.full(nt, GR, dtype=np.int64)
            ends = np.zeros(nt, dtype=np.int64)
            for pc in per_core:
                dd = pc[(g, s)][1]
                for t in range(nt):
                    seg = dd[t * CAP:(t + 1) * CAP]
                    if len(seg):
                        starts[t] = min(starts[t], seg[0])
                        ends[t] = max(ends[t], seg[-1] + 1)
            starts = np.minimum(starts, ends)  # empty tiles -> band at end
            bw = max(bw, int((ends - starts).max(initial=0)))
            row.append(tuple(int(v) for v in starts))
        db.append(tuple(row))
    bw = math.ceil(bw / 16) * 16
    # clamp band starts so db+bw stays inside the group
    db = tuple(
        tuple(tuple(min(v, GR - bw) for v in row_s) for row_s in row)
        for row in db)
    plan = Plan(tc=tuple(tc), db=db, bw=bw)

    CH = cfg.chunk
    tp = [math.ceil(plan.t_stream(s) / CH) * CH for s in range(2)]

    wt = np.asarray(W).T.astype(np.float32)
    bb = np.tile(np.asarray(b)[None, :].astype(np.float32), (128, 1))

    def wrap_idx(idx_lin):
        return np.tile(np.ascontiguousarray(idx_lin.reshape(-1, 16).T), (8, 1))

    in_maps = []
    for c in range(cfg.n_cores):
        buckets = per_core[c]
        idx_lin = [np.zeros(tp[s] * CAP, dtype=np.int16) for s in range(2)]
        sval = [np.zeros((128, tp[s] * bw), dtype=np.float64) for s in range(2)]
        spos = [0, 0]
        for g in range(NG):
            for t in range(max(plan.tc[g])):
                for s in range(2):
                    if t >= plan.tc[g][s]:
                        continue
                    ss, dd, vv = buckets[(g, s)]
                    ss = ss[t * CAP:(t + 1) * CAP]
                    dd = dd[t * CAP:(t + 1) * CAP]
                    vv = vv[t * CAP:(t + 1) * CAP]
                    pos = spos[s]
                    base = plan.db[g][s][t]
                    idx_lin[s][pos * CAP:pos * CAP + len(ss)] = ss
                    np.add.at(sval[s],
                              (np.arange(len(ss)), pos * bw + dd - base), vv)
                    spos[s] += 1
        in_maps.append({
            "xplo": xplo, "xphi": xphi,
            "gilo": np.ascontiguousarray(wrap_idx(idx_lin[0])),
            "gihi": np.ascontiguousarray(wrap_idx(idx_lin[1])),
            "svlo": np.ascontiguousarray(
                sval[0].astype(np.float32).astype(
                    np.asarray(xb).dtype)),
            "svhi": np.ascontiguousarray(
                sval[1].astype(np.float32).astype(
                    np.asarray(xb).dtype)),
            "cst": np.ascontiguousarray(
                np.concatenate([bb, wt], axis=1), dtype=np.float32),
        })
    return plan, in_maps


def _build_nc(cfg: Cfg, plan: Plan):
    from contextlib import ExitStack

    import concourse.bacc as bacc
    import concourse.mybir as mybir
    import concourse.tile as tile

    f32 = mybir.dt.float32
    bf16 = mybir.dt.bfloat16
    i16 = mybir.dt.int16
    NG = cfg.n_groups
    RPC = cfg.rows_per_core
    CH = cfg.chunk
    SPLIT = cfg.split
    BW = plan.bw
    tp = [math.ceil(plan.t_stream(s) / CH) * CH for s in range(2)]

    CW = 2 * D
    O_BB, O_WT = 0, D

    nc = bacc.Bacc("TRN2", target_bir_lowering=False,
                   num_swdge_queues=cfg.gather_queues)
    xp = [nc.dram_tensor("xplo", [SPLIT, D], bf16, kind="ExternalInput"),
          nc.dram_tensor("xphi", [SPLIT, D], bf16, kind="ExternalInput")]
    gi = [nc.dram_tensor("gilo", [128, tp[0] * 8], i16, kind="ExternalInput"),
          nc.dram_tensor("gihi", [128, tp[1] * 8], i16, kind="ExternalInput")]
    sv = [nc.dram_tensor("svlo", [128, tp[0] * BW], bf16,
                         kind="ExternalInput"),
          nc.dram_tensor("svhi", [128, tp[1] * BW], bf16,
                         kind="ExternalInput")]
    cst = nc.dram_tensor("cst", [128, CW], f32, kind="ExternalInput")
    y = nc.dram_tensor("y", [NG * GR, D], f32, kind="ExternalOutput")

    with tile.TileContext(nc) as tc, ExitStack() as ctx:
        const = ctx.enter_context(tc.tile_pool(name="const", bufs=1))
        gpool = [ctx.enter_context(tc.tile_pool(name="glo",
                                                bufs=cfg.gather_bufs)),
                 ctx.enter_context(tc.tile_pool(name="ghi",
                                                bufs=cfg.gather_bufs))]
        epool = ctx.enter_context(tc.tile_pool(name="epilog", bufs=2))
        ps_agg = ctx.enter_context(tc.tile_pool(name="psagg", bufs=2,
                                                space="PSUM"))
        ps_out = ctx.enter_context(tc.tile_pool(name="psout", bufs=2,
                                                space="PSUM"))

        cst_sb = const.tile([128, CW], f32)
        nc.sync.dma_start(out=cst_sb[:], in_=cst[:])

        def bb_ap(p):
            return cst_sb[:p, O_BB:O_BB + D]

        def wt_ap():
            return cst_sb[:, O_WT:O_WT + D]

        gi_sb = []
        sv_sb = []
        for s in range(2):
            t = const.tile([128, tp[s] * 8], i16, tag=f"gi{s}")
            nc.sync.dma_start(out=t[:], in_=gi[s][:])
            gi_sb.append(t)
            t = const.tile([128, tp[s] * BW], bf16, tag=f"sv{s}")
            nc.sync.dma_start(out=t[:], in_=sv[s][:])
            sv_sb.append(t)

        gbuf = [None, None]

        def fetch_chunk(s, ci):
            gbuf[s] = gpool[s].tile([128, CH * D], bf16, tag=f"gb{s}",
                                    name=f"gbuf{s}")
            nc.gpsimd.dma_gather(
                gbuf[s][:].rearrange("p (k j) -> p k j", j=D),
                xp[s][:],
                gi_sb[s][:, ci * CH * 8:(ci + 1) * CH * 8],
                CH * 128, CH * 128, D,
                single_packet=(CH * 128 <= 1024),
                queue_num=(s * 2 + ci % 2 if cfg.gather_queues == 4
                           else s % cfg.gather_queues),
            )

        def body():
            if cfg.skip_compute:
                for ci in range(max(tp) // CH):
                    for s in range(2):
                        if ci < tp[s] // CH:
                            fetch_chunk(s, ci)
                return
            spos = [0, 0]
            for g in range(NG):
                rows_g = min(GR, RPC - g * GR)
                agg = ps_agg.tile([128, GR], f32)
                first = True
                for t in range(max(plan.tc[g])):
                    for s in range(2):
                        if t >= plan.tc[g][s]:
                            continue
                        if spos[s] % CH == 0:
                            fetch_chunk(s, spos[s] // CH)
                        k = spos[s] % CH
                        base = plan.db[g][s][t]
                        nc.tensor.matmul(
                            out=agg[:, base:base + BW],
                            lhsT=gbuf[s][:, k * D:(k + 1) * D],
                            rhs=sv_sb[s][:, spos[s] * BW:(spos[s] + 1) * BW],
                            start=first, stop=True,
                            skip_group_check=True,
                        )
                        first = False
                        spos[s] += 1
                agg_sb = epool.tile([128, GR], f32, tag="aggsb")
                nc.vector.tensor_copy(out=agg_sb[:, :rows_g],
                                      in_=agg[:, :rows_g])
                for rc in range(math.ceil(rows_g / 128)):
                    w = min(128, rows_g - rc * 128)
                    out_ps = ps_out.tile([128, D], f32)
                    nc.tensor.matmul(
                        out=out_ps[:w, :],
                        lhsT=agg_sb[:, rc * 128:rc * 128 + w],
                        rhs=wt_ap(),
                        start=True, stop=True,
                    )
                    out_sb = epool.tile([128, D], f32, tag="outsb")
                    nc.vector.tensor_tensor(
                        out=out_sb[:w, :], in0=out_ps[:w, :], in1=bb_ap(w),
                        op=mybir.AluOpType.add,
                    )
                    r0 = g * GR + rc * 128
                    nc.sync.dma_start(out=y[r0:r0 + w, :], in_=out_sb[:w, :])

        if cfg.loop_n > 0:
            with tc.For_i(0, cfg.loop_n, 1):
                body()
        else:
            for _ in range(cfg.repeats):
                body()

    nc.compile()
    return nc


_CACHE = {}


def _get_nc(cfg: Cfg, plan: Plan):
    key = (cfg, plan)
    if key not in _CACHE:
        _CACHE[key] = _build_nc(cfg, plan)
    return _CACHE[key]


def kernel(x, edge_row, edge_col, edge_val, W, b):
    from concourse.bass_utils import run_bass_kernel_spmd

    cfg = Cfg()
    plan, in_maps = _preprocess(cfg, x, edge_row, edge_col, edge_val, W, b)
    nc = _get_nc(cfg, plan)
    res = run_bass_kernel_spmd(nc, in_maps, core_ids=list(range(cfg.n_cores)))
    RPC = cfg.rows_per_core
    out = np.empty((cfg.n_nodes, D), dtype=np.float32)
    for c in range(cfg.n_cores):
        out[c * RPC:(c + 1) * RPC] = res.results[c]["y"][:RPC]
    return out
